# revision 32
# baseline (speedup 1.0000x reference)
"""Trainium2 Bass kernel for per-token grouped attention (GQA-style).

Computation (per token t):
    q = x @ Wq.T + bq ; k = x @ Wk.T + bk ; v = x @ Wv.T + bv     (D=2048)
    reshape to (G=16 groups, d=128); scores = q_g . k_h / sqrt(d) (16x16)
    att = softmax(scores, axis=h); out = att @ v  -> (G*d,)

Sharding: data-parallel over the B*T = 16384 tokens across 8 cores
(2048 tokens/core).  Everything on-device is feature-major ("transposed")
so that the PE contracts over the partition axis; the host transposes x
on the way in and unscrambles the output on the way out.

Device program (per core, SPMD):
  Phase 1 (projections): qT/kT/vT = W.T-tiles @ xT, bf16 matmuls with
    fp32 PSUM accumulation.  m-tiles are processed in PAIRS with the two
    accumulation chains interleaved across two PSUM banks so consecutive
    matmuls never accumulate into the same bank region (same-bank
    accumulation serializes fill-after-drain and costs ~43ns/matmul).
    Bias is added during the PSUM->SBUF copy (ACT/DVE alternating).
  Phase 2 (attention): tokens processed in blocks of 8; one 128x128
    matmul computes all 64 pairwise 16x16 score tiles of an 8-token
    block (only the 8 diagonal tiles are kept - masked softmax), then a
    block-diagonal trick turns att @ v into another 128x128 matmul after
    two PE transposes.  Output is written bf16 in a DMA-friendly slab
    layout, one small store per 32-token super-block (no big end-of-
    kernel store bubble), and unscrambled + upcast on the host.
"""

import os
import numpy as np
import ml_dtypes

import concourse.bass as bass
import concourse.tile as tile
from concourse import bacc, mybir
from concourse.bass_utils import run_bass_kernel_spmd

F32 = mybir.dt.float32
BF16 = mybir.dt.bfloat16
AF = mybir.ActivationFunctionType
ALU = mybir.AluOpType

P = 128          # SBUF partitions
D = 2048         # model dim
G = 16           # groups
DG = 128         # per-group dim
N_CORES = 8
TC = 2048        # tokens per core
NCHUNK = 4      # phase-1 token chunks
CH = TC // NCHUNK          # 512
NTILE = 8       # phase-2 token tiles
TT = TC // NTILE           # 256
NSB = TT // 32  # super-blocks per tile (4 blocks of 8 tokens each) = 8
KT = D // P      # 16 contraction tiles
MT = D // P      # 16 output-feature tiles


def _emit(nc, tc, ctx):
    # ---- DRAM I/O -------------------------------------------------------
    xT = nc.dram_tensor("xT", [D, TC], BF16, kind="ExternalInput").ap()
    wT = {
        p: nc.dram_tensor(f"w{p}T", [D, D], BF16, kind="ExternalInput").ap()
        for p in "qkv"
    }
    b_dram = {
        p: nc.dram_tensor(f"b{p}", [P, G], F32, kind="ExternalInput").ap()
        for p in "qkv"
    }
    m01_dram = nc.dram_tensor("m01", [P, 4, P], BF16, kind="ExternalInput").ap()
    ident_dram = nc.dram_tensor("ident", [P, P], BF16, kind="ExternalInput").ap()
    # output: bf16 slabs [dd, tile, sb, g, 32-token run]; host unscrambles
    outT = nc.dram_tensor("outT", [P, NTILE, NSB, G, 32], BF16,
                          kind="ExternalOutput").ap()

    # ---- pools ----------------------------------------------------------
    singles = ctx.enter_context(tc.tile_pool(name="singles", bufs=1))
    xpool = ctx.enter_context(tc.tile_pool(name="xpool", bufs=2))
    wpool = ctx.enter_context(tc.tile_pool(name="wpool", bufs=4))
    pp_ps = ctx.enter_context(tc.tile_pool(name="pp_ps", bufs=4, space="PSUM"))
    asmp = ctx.enter_context(tc.tile_pool(name="asmp", bufs=2))

    obp = ctx.enter_context(tc.tile_pool(name="obp", bufs=4))
    smallp = ctx.enter_context(tc.tile_pool(name="smallp", bufs=2))
    attp = ctx.enter_context(tc.tile_pool(name="attp", bufs=2))
    # all 16 attT tiles of the last chunk are alive at once (A pieces run
    # during the v-projection, B pieces drain after it) — 16 bufs, 2MB.
    atp = ctx.enter_context(tc.tile_pool(name="atp", bufs=16))
    vtp = ctx.enter_context(tc.tile_pool(name="vtp", bufs=4))
    ps_s = ctx.enter_context(tc.tile_pool(name="ps_s", bufs=1, space="PSUM"))
    # att- and v-transposes share one PSUM ring (they strictly alternate)
    ps_tr = ctx.enter_context(tc.tile_pool(name="ps_tr", bufs=2, space="PSUM"))
    ps_o = ctx.enter_context(tc.tile_pool(name="ps_o", bufs=1, space="PSUM"))

    # DRAM views
    xT_v = xT.rearrange("(k p) t -> p k t", p=P)          # [P, KT, TC]
    wT_v = {p: wT[p].rearrange("(k p) o -> p k o", p=P) for p in "qkv"}

    # ---- startup ---------------------------------------------------------
    # DMA queue dispatch costs ~600ns per descriptor, so the startup spreads
    # across three queues: weights on sync, x on gpsimd (sw-DGE), consts on
    # scalar.  The first q-weight pair streams in k-grouped slices so the
    # first matmul can start as soon as w[k=0..3] + x[k=0..3] land.
    bias_sb = {}
    for p in "qkv":
        bias_sb[p] = singles.tile([P, G], F32, tag=f"bias{p}", name=f"bias{p}")
        nc.scalar.dma_start(out=bias_sb[p][:], in_=b_dram[p][:])
    xt0 = xpool.tile([P, KT, CH], BF16, tag="xt", name="xt")
    for kq in range(4):
        nc.gpsimd.dma_start(out=xt0[:, 4 * kq:4 * kq + 4, :],
                            in_=xT_v[:, 4 * kq:4 * kq + 4, 0:CH])
    # first two q-weight pairs in k-grouped slices: few enough dispatches
    # (~600ns each) to not serialize the queue, fine enough that the first
    # chain's k=0 step can start ~2us in
    w_first = wpool.tile([P, KT, 2 * P], BF16, tag="wt", name="wt")
    for kq in range(4):
        nc.sync.dma_start(out=w_first[:, 4 * kq:4 * kq + 4, :],
                          in_=wT_v["q"][:, 4 * kq:4 * kq + 4, 0:2 * P])
    w_second = wpool.tile([P, KT, 2 * P], BF16, tag="wt", name="wt")
    for kh in range(2):
        nc.sync.dma_start(out=w_second[:, 8 * kh:8 * kh + 8, :],
                          in_=wT_v["q"][:, 8 * kh:8 * kh + 8, 2 * P:4 * P])
    # HAM warm-up: the PE clock-gate only opens after ~3.4us of sustained
    # matmul activity.  While the startup DMAs land, run a burst of dummy
    # matmuls (never read back) on the just-landed bias tiles so the real
    # projection matmuls run at 2.4GHz from the first pair instead of
    # warming up ~30us in.
    warm_ps = pp_ps.tile([P, CH], F32, tag="pp", name="warm")
    for _ in range(18):
        nc.tensor.matmul(warm_ps[:, 0:2 * P], lhsT=w_first[:, 0, 0:P],
                         rhs=w_first[:, 0, :], start=True, stop=True)

    # mask + identity are only needed once attention pieces start (during
    # chunk 1); load them lazily so they don't delay the startup weights.
    m01_sb = singles.tile([P, 4, P], BF16, tag="m01", name="m01")
    ident_sb = singles.tile([P, P], BF16, tag="ident", name="ident")

    def filler():
        f_ps = ps_o.tile([P, 4, P], F32, tag="o", name="fill")
        nc.tensor.matmul(f_ps[:, 0, :], lhsT=m01_sb[:, 0, :],
                         rhs=m01_sb[:, 0, :], start=True, stop=True)

    def load_consts():
        nc.scalar.dma_start(out=m01_sb[:], in_=m01_dram[:])
        nc.scalar.dma_start(out=ident_sb[:], in_=ident_dram[:])

    # assembled q/k/v chunk tiles stay resident in SBUF (block-interleaved
    # [dd, block, g, s]); attention reads them directly - no DRAM round-trip.
    chunk_asm = {}

    # ---- attention emission pieces -------------------------------------
    # Each token tile yields 8 A-pieces (scores MMs + softmax chain) and 8
    # B-pieces (v-transpose + att@v + slab store).  Pieces are pumped into
    # the projection emission of the NEXT chunk so DVE/ACT softmax work
    # hides under projection matmuls and the PE never waits on it.
    def make_tile_pieces(t, drain=False):
        st = {}
        c, half = t // (CH // TT), t % (CH // TT)
        nb = TT // 8

        def prologue():
            st["att"] = {}
            st["vT"] = {}

        def piece_a(sb):
            q2f = chunk_asm[c]["q"].rearrange("p b g s -> p (b g s)")
            k2f = chunk_asm[c]["k"].rearrange("p b g s -> p (b g s)")
            s_ps = ps_s.tile([P, 4, P], F32, tag="s", name="s")
            for j in range(4):
                b = half * nb + sb * 4 + j
                sl = slice(b * P, (b + 1) * P)
                nc.tensor.matmul(s_ps[:, j, :], lhsT=q2f[:, sl], rhs=k2f[:, sl],
                                 start=True, stop=True)
            # masked softmax over the 16-wide diagonal tiles, in bf16 so the
            # DVE runs at 2x throughput (16 of these chains must fit under
            # the last chunk's v-projection); sums accumulate in fp32
            e = smallp.tile([P, 4, P], BF16, tag="e", name="e")
            nc.scalar.activation(out=e[:], in_=s_ps[:], func=AF.Exp)
            em = smallp.tile([P, 4, P], BF16, tag="em", name="em")
            nc.vector.tensor_tensor(out=em[:], in0=e[:], in1=m01_sb[:],
                                    op=ALU.mult)
            sums = smallp.tile([P, 4], F32, tag="sums", name="sums")
            nc.vector.tensor_reduce(out=sums[:], in_=em[:],
                                    axis=mybir.AxisListType.X, op=ALU.add)
            rs = smallp.tile([P, 4], F32, tag="rs", name="rs")
            nc.vector.reciprocal(out=rs[:], in_=sums[:])
            att = attp.tile([P, 4, P], BF16, tag="att", name="att")
            for j in range(4):
                nc.vector.tensor_scalar_mul(att[:, j, :], em[:, j, :],
                                            rs[:, j:j + 1])
            a_ps = ps_tr.tile([P, 4, P], BF16, tag="tr", name="a")
            for j in range(4):
                nc.tensor.transpose(a_ps[:, j, :], att[:, j, :], ident_sb[:])
            attT = atp.tile([P, 4, P], BF16, tag="attT", name="attT")
            nc.scalar.copy(out=attT[:, 0:2, :], in_=a_ps[:, 0:2, :])
            nc.vector.tensor_copy(out=attT[:, 2:4, :], in_=a_ps[:, 2:4, :])
            st["att"][sb] = attT

        def piece_btr(sb):
            v2f = chunk_asm[c]["v"].rearrange("p b g s -> p (b g s)")
            # transpose v blocks: [d, (s,h)] -> [(s,h), d].  In the drain,
            # borrow the (idle) scores bank as a third transpose slot and
            # alternate whole-vT copies between ACT and DVE so the PE never
            # waits on a PSUM->SBUF copy.
            if drain and sb % 3 == 2:
                v_ps = ps_s.tile([P, 4, P], BF16, tag="s", name="v")
            else:
                v_ps = ps_tr.tile([P, 4, P], BF16, tag="tr", name="v")
            for j in range(4):
                b = half * nb + sb * 4 + j
                nc.tensor.transpose(v_ps[:, j, :], v2f[:, b * P:(b + 1) * P],
                                    ident_sb[:])
            vT = vtp.tile([P, 4, P], BF16, tag="vT", name="vT")
            if drain:
                eng = nc.scalar if sb % 2 == 0 else nc.vector
                (eng.copy if eng is nc.scalar else eng.tensor_copy)(
                    out=vT[:], in_=v_ps[:])
            else:
                nc.scalar.copy(out=vT[:, 0:1, :], in_=v_ps[:, 0:1, :])
                nc.vector.tensor_copy(out=vT[:, 1:4, :], in_=v_ps[:, 1:4, :])
            st["vT"][sb] = vT

        def piece_bmm(sb):
            attT = st["att"].pop(sb)
            vT = st["vT"].pop(sb)
            # att @ v -> out^T block [d, (s,g)].  In the drain, the idle
            # projection PSUM ring (4 banks) holds the outputs so four
            # pieces can be in flight.
            if drain:
                o_ps = pp_ps.tile([P, 4, P], F32, tag="pp", name="o")
            else:
                o_ps = ps_o.tile([P, 4, P], F32, tag="o", name="o")
            for j in range(4):
                nc.tensor.matmul(o_ps[:, j, :], lhsT=vT[:, j, :],
                                 rhs=attT[:, j, :], start=True, stop=True)
            # cast + regroup to [dd, g, 32-token run] and store the slab
            ob = obp.tile([P, G, 32], BF16, tag="ob", name="ob")
            dst = ob[:].rearrange("p g (j s) -> p g j s", j=4)
            src = o_ps[:].rearrange("p j (g s) -> p g j s", g=G)
            if drain:
                dst_lo = ob[:, 0:8, :].rearrange("p g (j s) -> p g j s", j=4)
                src_lo = o_ps[:, :, 0:64].rearrange("p j (g s) -> p g j s",
                                                    g=8)
                nc.scalar.copy(out=dst_lo, in_=src_lo)
                dst_hi = ob[:, 8:16, :].rearrange("p g (j s) -> p g j s", j=4)
                src_hi = o_ps[:, :, 64:128].rearrange("p j (g s) -> p g j s",
                                                      g=8)
                nc.vector.tensor_copy(out=dst_hi, in_=src_hi)
                sq = nc.sync if sb % 2 == 0 else nc.gpsimd
                sq.dma_start(out=outT[:, t, sb], in_=ob[:])
            else:
                dst_lo = ob[:, 0:8, :].rearrange("p g (j s) -> p g j s", j=4)
                src_lo = o_ps[:, :, 0:64].rearrange("p j (g s) -> p g j s", g=8)
                nc.scalar.copy(out=dst_lo, in_=src_lo)
                dst_hi = ob[:, 8:16, :].rearrange("p g (j s) -> p g j s", j=4)
                src_hi = o_ps[:, :, 64:128].rearrange("p j (g s) -> p g j s",
                                                      g=8)
                nc.vector.tensor_copy(out=dst_hi, in_=src_hi)
                nc.gpsimd.dma_start(out=outT[:, t, sb], in_=ob[:])

        a = [lambda sb=sb: piece_a(sb) for sb in range(NSB)]
        btr = [lambda sb=sb: piece_btr(sb) for sb in range(NSB)]
        bmm = [lambda sb=sb: piece_bmm(sb) for sb in range(NSB)]
        return prologue, a, btr, bmm

    def interleave_ab(prologues, aa, btr, bmm):
        """A(i) leads; Btr lags A by 2; Bmm lags Btr by 1 so the out-matmuls
        never sit in the PE FIFO waiting on the vT PSUM->SBUF copy."""
        out = list(prologues)
        n = len(aa)
        for i in range(n + 3):
            if i < n:
                out.append(aa[i])
            if 2 <= i < n + 2:
                out.append(btr[i - 2])
            if 3 <= i:
                out.append(bmm[i - 3])
        return out

    def chunk_pieces(c):
        """Pieces for the two token tiles computed in chunk c."""
        t0, t1 = 2 * c, 2 * c + 1
        p0, a0, btr0, bmm0 = make_tile_pieces(t0)
        p1, a1, btr1, bmm1 = make_tile_pieces(t1)
        return interleave_ab([p0, p1], a0 + a1, btr0 + btr1, bmm0 + bmm1)

    def last_chunk_pieces(c):
        """Split: [prologues + A pieces] pumped into this chunk's own
        v-projection; [B pieces, lag-interleaved] drain at the end."""
        t0, t1 = 2 * c, 2 * c + 1
        p0, a0, btr0, bmm0 = make_tile_pieces(t0, drain=True)
        p1, a1, btr1, bmm1 = make_tile_pieces(t1, drain=True)
        pre = [p0, p1] + a0 + a1
        btr = btr0 + btr1
        bmm = bmm0 + bmm1
        post = []
        for i in range(len(btr) + 1):
            if i < len(btr):
                post.append(btr[i])
            if i >= 1:
                post.append(bmm[i - 1])
        return pre, post

    # ---- phase 1: projections with attention pieces pumped in ----------
    def load_x(c):
        xt = xpool.tile([P, KT, CH], BF16, tag="xt", name="xt")
        nc.gpsimd.dma_start(out=xt[:], in_=xT_v[:, :, c * CH:(c + 1) * CH])
        return xt

    pending = []
    post_pieces = []
    xts = {0: xt0}
    for c in range(NCHUNK):
        xt = xts.pop(c)
        for p in "qkv":
            # prefetch the next x chunk AFTER the q weights so it doesn't
            # delay the weight stream at startup
            if p == "k" and c + 1 < NCHUNK:
                xts[c + 1] = load_x(c + 1)
            if c == NCHUNK - 1 and p == "v":
                pre, post_pieces = last_chunk_pieces(c)
                pending.extend(pre)
            asm = asmp.tile([P, CH // 8, G, 8], BF16, tag=f"asm{p}",
                            name=f"asm{p}")
            chunk_asm.setdefault(c, {})[p] = asm
            for mp in range(MT // 2):
                m0, m1 = 2 * mp, 2 * mp + 1
                if c == 0 and p == "q" and mp == 2:
                    load_consts()
                if c == 0 and p == "q" and mp == 0:
                    w = w_first
                elif c == 0 and p == "q" and mp == 1:
                    w = w_second
                else:
                    w = wpool.tile([P, KT, 2 * P], BF16, tag="wt", name="wt")
                    nc.sync.dma_start(
                        out=w[:], in_=wT_v[p][:, :, m0 * P:(m0 + 2) * P])
                # two accumulation chains interleaved across two PSUM banks
                ps0 = pp_ps.tile([P, CH], F32, tag="pp", name="pp")
                ps1 = pp_ps.tile([P, CH], F32, tag="pp", name="pp")
                for k in range(KT):
                    nc.tensor.matmul(ps0[:], lhsT=w[:, k, 0:P],
                                     rhs=xt[:, k, :],
                                     start=(k == 0), stop=(k == KT - 1))
                    nc.tensor.matmul(ps1[:], lhsT=w[:, k, P:2 * P],
                                     rhs=xt[:, k, :],
                                     start=(k == 0), stop=(k == KT - 1))
                # bias + cast + scatter into the interleaved layout; alternate
                # ACT/DVE so neither engine's queue delays the attention chain
                for m, ps in ((m0, ps0), (m1, ps1)):
                    dst = asm[:, :, m, :]
                    src = ps[:].rearrange("p (b s) -> p b s", s=8)
                    if m % 2 == 0:
                        nc.scalar.activation(out=dst, in_=src, func=AF.Identity,
                                             bias=bias_sb[p][:, m:m + 1],
                                             scale=1.0)
                    else:
                        nc.vector.tensor_scalar_add(dst, src,
                                                    bias_sb[p][:, m:m + 1])
                if pending:
                    pending.pop(0)()
                    if len(pending) > 6:
                        pending.pop(0)()
                    if len(pending) > 8:
                        pending.pop(0)()
                    if len(pending) > 12:
                        pending.pop(0)()
        if c < NCHUNK - 1:
            pending.extend(chunk_pieces(c))

    # drain the last chunk's attention (B pieces); filler matmuls keep the
    # PE activity monitor from re-throttling the clock to 1.2GHz
    for piece in pending + post_pieces:
        piece()
        filler()


_PROGRAM = None


def _build():
    global _PROGRAM
    if _PROGRAM is not None:
        return _PROGRAM
    from contextlib import ExitStack

    nc = bacc.Bacc("TRN2", target_bir_lowering=False, debug=False,
                   num_devices=N_CORES)
    with tile.TileContext(nc) as tc:
        with ExitStack() as ctx:
            _emit(nc, tc, ctx)
    nc.compile()
    _PROGRAM = nc
    return nc


def _host_inputs(x, Wq, bq, Wk, bk, Wv, bv):
    """Build the per-core input maps (host-side shard + transpose + cast)."""
    scale = 1.0 / np.sqrt(DG)
    xf = np.ascontiguousarray(x.reshape(-1, D))           # [16384, D]
    assert xf.shape[0] == N_CORES * TC

    bf = ml_dtypes.bfloat16
    shared = {
        "wqT": np.ascontiguousarray((Wq * scale).T).astype(bf),
        "wkT": np.ascontiguousarray(Wk.T).astype(bf),
        "wvT": np.ascontiguousarray(Wv.T).astype(bf),
        "bq": np.ascontiguousarray((bq * scale).reshape(G, DG).T).astype(np.float32),
        "bk": np.ascontiguousarray(bk.reshape(G, DG).T).astype(np.float32),
        "bv": np.ascontiguousarray(bv.reshape(G, DG).T).astype(np.float32),
        "m01": np.ascontiguousarray(np.broadcast_to(
            np.kron(np.ones((G, G), dtype=np.float32),
                    np.eye(8, dtype=np.float32))[:, None, :],
            (P, 4, P))).astype(bf),
        "ident": np.eye(P, dtype=np.float32).astype(bf),
    }
    in_maps = []
    for i in range(N_CORES):
        xi = xf[i * TC:(i + 1) * TC]
        m = dict(shared)
        m["xT"] = np.ascontiguousarray(xi.T).astype(bf)
        in_maps.append(m)
    return in_maps


last_results = None


def _install_ntff_shim():
    """Provide antenv.axon_hooks if the image lacks it (profiling only)."""
    import sys
    try:
        from antenv.axon_hooks import get_axon_ntff_profile_hook  # noqa: F401
        return
    except ImportError:
        pass
    import contextlib
    import ctypes
    import types

    so_path = "/opt/axon/libaxon_pjrt.so"
    hook = None
    if os.path.exists(so_path):
        lib = ctypes.CDLL(so_path)
        if hasattr(lib, "axon_start_nrt_profile"):
            lib.axon_start_nrt_profile.argtypes = [
                ctypes.POINTER(ctypes.c_int64), ctypes.c_size_t]
            lib.axon_start_nrt_profile.restype = ctypes.c_int64
            lib.axon_stop_nrt_profile.argtypes = [ctypes.c_char_p]
            lib.axon_stop_nrt_profile.restype = ctypes.c_int64

            @contextlib.contextmanager
            def _hook(output_dir, device_ids):
                import jax
                jax.devices()
                if device_ids:
                    ids = (ctypes.c_int64 * len(device_ids))(*device_ids)
                    rc = lib.axon_start_nrt_profile(ids, len(device_ids))
                else:
                    rc = lib.axon_start_nrt_profile(None, 0)
                if rc != 0:
                    raise RuntimeError(f"axon_start_nrt_profile rc={rc}")
                try:
                    yield
                finally:
                    n = lib.axon_stop_nrt_profile(str(output_dir).encode())
                    print(f"profile: {n} file(s) written to {output_dir}")

            hook = _hook

    mod = types.ModuleType("antenv.axon_hooks")
    mod.get_axon_ntff_profile_hook = lambda: hook
    mod.set_axon_ntff_profile_hook = lambda h: None
    import antenv
    antenv.axon_hooks = mod
    sys.modules["antenv.axon_hooks"] = mod


def kernel(**inputs):
    global last_results
    nc = _build()
    in_maps = _host_inputs(**inputs)
    trace = bool(os.environ.get("BASS_TRACE"))
    if trace:
        _install_ntff_shim()
    res = run_bass_kernel_spmd(nc, in_maps, list(range(N_CORES)), trace=trace)
    last_results = res
    x = inputs["x"]
    out = np.empty((N_CORES * TC, D), dtype=np.float32)
    for i in range(N_CORES):
        # [dd, tile, sb, g, 32] -> [tile, sb, 32, g, dd] -> [TC, D]
        ot = res.results[i]["outT"].astype(np.float32)
        out[i * TC:(i + 1) * TC] = (
            ot.transpose(1, 2, 4, 3, 0).reshape(TC, D))
    return out.reshape(x.shape)


# revision 35
# speedup vs baseline: 1.0110x; 1.0110x over previous
"""Trainium2 Bass kernel for per-token grouped attention (GQA-style).

Computation (per token t):
    q = x @ Wq.T + bq ; k = x @ Wk.T + bk ; v = x @ Wv.T + bv     (D=2048)
    reshape to (G=16 groups, d=128); scores = q_g . k_h / sqrt(d) (16x16)
    att = softmax(scores, axis=h); out = att @ v  -> (G*d,)

Sharding: data-parallel over the B*T = 16384 tokens across 8 cores
(2048 tokens/core).  Everything on-device is feature-major ("transposed")
so that the PE contracts over the partition axis; the host transposes x
on the way in and unscrambles the output on the way out.

Device program (per core, SPMD):
  Phase 1 (projections): qT/kT/vT = W.T-tiles @ xT, bf16 matmuls with
    fp32 PSUM accumulation.  m-tiles are processed in PAIRS with the two
    accumulation chains interleaved across two PSUM banks so consecutive
    matmuls never accumulate into the same bank region (same-bank
    accumulation serializes fill-after-drain and costs ~43ns/matmul).
    Bias is added during the PSUM->SBUF copy (ACT/DVE alternating).
  Phase 2 (attention): tokens processed in blocks of 8; one 128x128
    matmul computes all 64 pairwise 16x16 score tiles of an 8-token
    block (only the 8 diagonal tiles are kept - masked softmax), then a
    block-diagonal trick turns att @ v into another 128x128 matmul after
    two PE transposes.  Output is written bf16 in a DMA-friendly slab
    layout, one small store per 32-token super-block (no big end-of-
    kernel store bubble), and unscrambled + upcast on the host.
"""

import os
import numpy as np
import ml_dtypes

import concourse.bass as bass
import concourse.tile as tile
from concourse import bacc, mybir
from concourse.bass_utils import run_bass_kernel_spmd

F32 = mybir.dt.float32
BF16 = mybir.dt.bfloat16
AF = mybir.ActivationFunctionType
ALU = mybir.AluOpType

P = 128          # SBUF partitions
D = 2048         # model dim
G = 16           # groups
DG = 128         # per-group dim
N_CORES = 8
TC = 2048        # tokens per core
NCHUNK = 4      # phase-1 token chunks
CH = TC // NCHUNK          # 512
NTILE = 8       # phase-2 token tiles
TT = TC // NTILE           # 256
NSB = TT // 32  # super-blocks per tile (4 blocks of 8 tokens each) = 8
KT = D // P      # 16 contraction tiles
MT = D // P      # 16 output-feature tiles


def _emit(nc, tc, ctx):
    # ---- DRAM I/O -------------------------------------------------------
    xT = nc.dram_tensor("xT", [D, TC], BF16, kind="ExternalInput").ap()
    wT = {
        p: nc.dram_tensor(f"w{p}T", [D, D], BF16, kind="ExternalInput").ap()
        for p in "qkv"
    }
    b_dram = {
        p: nc.dram_tensor(f"b{p}", [P, G], F32, kind="ExternalInput").ap()
        for p in "qkv"
    }
    m01_dram = nc.dram_tensor("m01", [P, 4, P], BF16, kind="ExternalInput").ap()
    ident_dram = nc.dram_tensor("ident", [P, P], BF16, kind="ExternalInput").ap()
    # output: bf16 slabs [dd, tile, sb, g, 32-token run]; host unscrambles
    outT = nc.dram_tensor("outT", [P, NTILE, NSB, G, 32], BF16,
                          kind="ExternalOutput").ap()

    # ---- pools ----------------------------------------------------------
    singles = ctx.enter_context(tc.tile_pool(name="singles", bufs=1))
    xpool = ctx.enter_context(tc.tile_pool(name="xpool", bufs=2))
    wpool = ctx.enter_context(tc.tile_pool(name="wpool", bufs=4))
    pp_ps = ctx.enter_context(tc.tile_pool(name="pp_ps", bufs=4, space="PSUM"))
    asmp = ctx.enter_context(tc.tile_pool(name="asmp", bufs=2))

    obp = ctx.enter_context(tc.tile_pool(name="obp", bufs=4))
    smallp = ctx.enter_context(tc.tile_pool(name="smallp", bufs=2))
    attp = ctx.enter_context(tc.tile_pool(name="attp", bufs=2))
    # all 16 attT tiles of the last chunk are alive at once (A pieces run
    # during the v-projection, B pieces drain after it) — 16 bufs, 2MB.
    atp = ctx.enter_context(tc.tile_pool(name="atp", bufs=16))
    vtp = ctx.enter_context(tc.tile_pool(name="vtp", bufs=4))
    ps_s = ctx.enter_context(tc.tile_pool(name="ps_s", bufs=1, space="PSUM"))
    # att- and v-transposes share one PSUM ring (they strictly alternate)
    ps_tr = ctx.enter_context(tc.tile_pool(name="ps_tr", bufs=2, space="PSUM"))
    ps_o = ctx.enter_context(tc.tile_pool(name="ps_o", bufs=1, space="PSUM"))

    # DRAM views
    xT_v = xT.rearrange("(k p) t -> p k t", p=P)          # [P, KT, TC]
    wT_v = {p: wT[p].rearrange("(k p) o -> p k o", p=P) for p in "qkv"}

    # ---- startup ---------------------------------------------------------
    # DMA queue dispatch costs ~600ns per descriptor, so the startup spreads
    # across three queues: weights on sync, x on gpsimd (sw-DGE), consts on
    # scalar.  The first q-weight pair streams in k-grouped slices so the
    # first matmul can start as soon as w[k=0..3] + x[k=0..3] land.
    bias_sb = {}
    for p in "qkv":
        bias_sb[p] = singles.tile([P, G], F32, tag=f"bias{p}", name=f"bias{p}")
        nc.scalar.dma_start(out=bias_sb[p][:], in_=b_dram[p][:])
    xt0 = xpool.tile([P, KT, CH], BF16, tag="xt", name="xt")
    for kq in range(4):
        nc.gpsimd.dma_start(out=xt0[:, 4 * kq:4 * kq + 4, :],
                            in_=xT_v[:, 4 * kq:4 * kq + 4, 0:CH])
    # first two q-weight pairs in k-grouped slices: few enough dispatches
    # (~600ns each) to not serialize the queue, fine enough that the first
    # chain's k=0 step can start ~2us in
    w_first = wpool.tile([P, KT, 2 * P], BF16, tag="wt", name="wt")
    for kq in range(4):
        nc.sync.dma_start(out=w_first[:, 4 * kq:4 * kq + 4, :],
                          in_=wT_v["q"][:, 4 * kq:4 * kq + 4, 0:2 * P])
    w_second = wpool.tile([P, KT, 2 * P], BF16, tag="wt", name="wt")
    for kh in range(2):
        nc.sync.dma_start(out=w_second[:, 8 * kh:8 * kh + 8, :],
                          in_=wT_v["q"][:, 8 * kh:8 * kh + 8, 2 * P:4 * P])
    # HAM warm-up: the PE clock-gate only opens after ~3.4us of sustained
    # matmul activity.  While the startup DMAs land, run a burst of dummy
    # matmuls (never read back) on the just-landed bias tiles so the real
    # projection matmuls run at 2.4GHz from the first pair instead of
    # warming up ~30us in.
    warm_ps = pp_ps.tile([P, CH], F32, tag="pp", name="warm")
    for _ in range(18):
        nc.tensor.matmul(warm_ps[:, 0:2 * P], lhsT=w_first[:, 0, 0:P],
                         rhs=w_first[:, 0, :], start=True, stop=True)

    # mask + identity are only needed once attention pieces start (during
    # chunk 1); load them lazily so they don't delay the startup weights.
    m01_sb = singles.tile([P, 4, P], BF16, tag="m01", name="m01")
    ident_sb = singles.tile([P, P], BF16, tag="ident", name="ident")

    def load_consts():
        nc.scalar.dma_start(out=m01_sb[:], in_=m01_dram[:])
        nc.scalar.dma_start(out=ident_sb[:], in_=ident_dram[:])

    # assembled q/k/v chunk tiles stay resident in SBUF (block-interleaved
    # [dd, block, g, s]); attention reads them directly - no DRAM round-trip.
    chunk_asm = {}

    # ---- attention emission pieces -------------------------------------
    # Each token tile yields 8 A-pieces (scores MMs + softmax chain) and 8
    # B-pieces (v-transpose + att@v + slab store).  Pieces are pumped into
    # the projection emission of the NEXT chunk so DVE/ACT softmax work
    # hides under projection matmuls and the PE never waits on it.
    def make_tile_pieces(t, drain=False):
        st = {}
        c, half = t // (CH // TT), t % (CH // TT)
        nb = TT // 8

        def prologue():
            st["att"] = {}
            st["vT"] = {}

        def piece_a(sb):
            q2f = chunk_asm[c]["q"].rearrange("p b g s -> p (b g s)")
            k2f = chunk_asm[c]["k"].rearrange("p b g s -> p (b g s)")
            s_ps = ps_s.tile([P, 4, P], F32, tag="s", name="s")
            for j in range(4):
                b = half * nb + sb * 4 + j
                sl = slice(b * P, (b + 1) * P)
                nc.tensor.matmul(s_ps[:, j, :], lhsT=q2f[:, sl], rhs=k2f[:, sl],
                                 start=True, stop=True)
            # masked softmax over the 16-wide diagonal tiles, in bf16 so the
            # DVE runs at 2x throughput (16 of these chains must fit under
            # the last chunk's v-projection); sums accumulate in fp32
            e = smallp.tile([P, 4, P], BF16, tag="e", name="e")
            nc.scalar.activation(out=e[:], in_=s_ps[:], func=AF.Exp)
            em = smallp.tile([P, 4, P], BF16, tag="em", name="em")
            nc.vector.tensor_tensor(out=em[:], in0=e[:], in1=m01_sb[:],
                                    op=ALU.mult)
            sums = smallp.tile([P, 4], F32, tag="sums", name="sums")
            nc.vector.tensor_reduce(out=sums[:], in_=em[:],
                                    axis=mybir.AxisListType.X, op=ALU.add)
            rs = smallp.tile([P, 4], F32, tag="rs", name="rs")
            nc.vector.reciprocal(out=rs[:], in_=sums[:])
            att = attp.tile([P, 4, P], BF16, tag="att", name="att")
            for j in range(4):
                nc.vector.tensor_scalar_mul(att[:, j, :], em[:, j, :],
                                            rs[:, j:j + 1])
            a_ps = ps_tr.tile([P, 4, P], BF16, tag="tr", name="a")
            for j in range(4):
                nc.tensor.transpose(a_ps[:, j, :], att[:, j, :], ident_sb[:])
            attT = atp.tile([P, 4, P], BF16, tag="attT", name="attT")
            nc.scalar.copy(out=attT[:, 0:2, :], in_=a_ps[:, 0:2, :])
            nc.vector.tensor_copy(out=attT[:, 2:4, :], in_=a_ps[:, 2:4, :])
            st["att"][sb] = attT

        def piece_btr(sb):
            v2f = chunk_asm[c]["v"].rearrange("p b g s -> p (b g s)")
            # transpose v blocks: [d, (s,h)] -> [(s,h), d].  In the drain,
            # borrow the (idle) scores bank as a third transpose slot and
            # alternate whole-vT copies between ACT and DVE so the PE never
            # waits on a PSUM->SBUF copy.
            if drain and sb % 3 == 2:
                v_ps = ps_s.tile([P, 4, P], BF16, tag="s", name="v")
            else:
                v_ps = ps_tr.tile([P, 4, P], BF16, tag="tr", name="v")
            for j in range(4):
                b = half * nb + sb * 4 + j
                nc.tensor.transpose(v_ps[:, j, :], v2f[:, b * P:(b + 1) * P],
                                    ident_sb[:])
            vT = vtp.tile([P, 4, P], BF16, tag="vT", name="vT")
            if drain:
                eng = nc.scalar if sb % 2 == 0 else nc.vector
                (eng.copy if eng is nc.scalar else eng.tensor_copy)(
                    out=vT[:], in_=v_ps[:])
            else:
                nc.scalar.copy(out=vT[:, 0:1, :], in_=v_ps[:, 0:1, :])
                nc.vector.tensor_copy(out=vT[:, 1:4, :], in_=v_ps[:, 1:4, :])
            st["vT"][sb] = vT

        def piece_bmm(sb):
            attT = st["att"].pop(sb)
            vT = st["vT"].pop(sb)
            # att @ v -> out^T block [d, (s,g)].  In the drain, the idle
            # projection PSUM ring (4 banks) holds the outputs so four
            # pieces can be in flight.
            if drain:
                o_ps = pp_ps.tile([P, 4, P], F32, tag="pp", name="o")
            else:
                o_ps = ps_o.tile([P, 4, P], F32, tag="o", name="o")
            for j in range(4):
                nc.tensor.matmul(o_ps[:, j, :], lhsT=vT[:, j, :],
                                 rhs=attT[:, j, :], start=True, stop=True)
            # cast + regroup to [dd, g, 32-token run] and store the slab
            ob = obp.tile([P, G, 32], BF16, tag="ob", name="ob")
            dst = ob[:].rearrange("p g (j s) -> p g j s", j=4)
            src = o_ps[:].rearrange("p j (g s) -> p g j s", g=G)
            if drain:
                eng = nc.vector if sb % 2 == 0 else nc.scalar
                (eng.copy if eng is nc.scalar else eng.tensor_copy)(
                    out=dst, in_=src)
                nc.sync.dma_start(out=outT[:, t, sb], in_=ob[:])
            else:
                dst_lo = ob[:, 0:8, :].rearrange("p g (j s) -> p g j s", j=4)
                src_lo = o_ps[:, :, 0:64].rearrange("p j (g s) -> p g j s", g=8)
                nc.scalar.copy(out=dst_lo, in_=src_lo)
                dst_hi = ob[:, 8:16, :].rearrange("p g (j s) -> p g j s", j=4)
                src_hi = o_ps[:, :, 64:128].rearrange("p j (g s) -> p g j s",
                                                      g=8)
                nc.vector.tensor_copy(out=dst_hi, in_=src_hi)
                nc.gpsimd.dma_start(out=outT[:, t, sb], in_=ob[:])

        a = [lambda sb=sb: piece_a(sb) for sb in range(NSB)]
        btr = [lambda sb=sb: piece_btr(sb) for sb in range(NSB)]
        bmm = [lambda sb=sb: piece_bmm(sb) for sb in range(NSB)]
        return prologue, a, btr, bmm

    def interleave_ab(prologues, aa, btr, bmm):
        """A(i) leads; Btr lags A by 2; Bmm lags Btr by 1 so the out-matmuls
        never sit in the PE FIFO waiting on the vT PSUM->SBUF copy."""
        out = list(prologues)
        n = len(aa)
        for i in range(n + 3):
            if i < n:
                out.append(aa[i])
            if 2 <= i < n + 2:
                out.append(btr[i - 2])
            if 3 <= i:
                out.append(bmm[i - 3])
        return out

    def chunk_pieces(c):
        """Pieces for the two token tiles computed in chunk c."""
        t0, t1 = 2 * c, 2 * c + 1
        p0, a0, btr0, bmm0 = make_tile_pieces(t0)
        p1, a1, btr1, bmm1 = make_tile_pieces(t1)
        return interleave_ab([p0, p1], a0 + a1, btr0 + btr1, bmm0 + bmm1)

    def last_chunk_pieces(c):
        """Split: [prologues + A pieces] pumped into this chunk's own
        v-projection; [B pieces, lag-interleaved] drain at the end."""
        t0, t1 = 2 * c, 2 * c + 1
        p0, a0, btr0, bmm0 = make_tile_pieces(t0, drain=True)
        p1, a1, btr1, bmm1 = make_tile_pieces(t1, drain=True)
        pre = [p0, p1] + a0 + a1
        btr = btr0 + btr1
        bmm = bmm0 + bmm1
        post = []
        for i in range(len(btr) + 1):
            if i < len(btr):
                post.append(btr[i])
            if i >= 1:
                post.append(bmm[i - 1])
        return pre, post

    # ---- phase 1: projections with attention pieces pumped in ----------
    def load_x(c):
        xt = xpool.tile([P, KT, CH], BF16, tag="xt", name="xt")
        nc.gpsimd.dma_start(out=xt[:], in_=xT_v[:, :, c * CH:(c + 1) * CH])
        return xt

    pending = []
    post_pieces = []
    xts = {0: xt0}
    for c in range(NCHUNK):
        xt = xts.pop(c)
        for p in "qkv":
            # prefetch the next x chunk AFTER the q weights so it doesn't
            # delay the weight stream at startup
            if p == "k" and c + 1 < NCHUNK:
                xts[c + 1] = load_x(c + 1)
            if c == NCHUNK - 1 and p == "v":
                pre, post_pieces = last_chunk_pieces(c)
                pending.extend(pre)
            asm = asmp.tile([P, CH // 8, G, 8], BF16, tag=f"asm{p}",
                            name=f"asm{p}")
            chunk_asm.setdefault(c, {})[p] = asm
            for mp in range(MT // 2):
                m0, m1 = 2 * mp, 2 * mp + 1
                if c == 0 and p == "q" and mp == 2:
                    load_consts()
                if c == 0 and p == "q" and mp == 0:
                    w = w_first
                elif c == 0 and p == "q" and mp == 1:
                    w = w_second
                else:
                    w = wpool.tile([P, KT, 2 * P], BF16, tag="wt", name="wt")
                    nc.sync.dma_start(
                        out=w[:], in_=wT_v[p][:, :, m0 * P:(m0 + 2) * P])
                # two accumulation chains interleaved across two PSUM banks
                ps0 = pp_ps.tile([P, CH], F32, tag="pp", name="pp")
                ps1 = pp_ps.tile([P, CH], F32, tag="pp", name="pp")
                for k in range(KT):
                    nc.tensor.matmul(ps0[:], lhsT=w[:, k, 0:P],
                                     rhs=xt[:, k, :],
                                     start=(k == 0), stop=(k == KT - 1))
                    nc.tensor.matmul(ps1[:], lhsT=w[:, k, P:2 * P],
                                     rhs=xt[:, k, :],
                                     start=(k == 0), stop=(k == KT - 1))
                # bias + cast + scatter into the interleaved layout; alternate
                # ACT/DVE so neither engine's queue delays the attention chain
                for m, ps in ((m0, ps0), (m1, ps1)):
                    dst = asm[:, :, m, :]
                    src = ps[:].rearrange("p (b s) -> p b s", s=8)
                    if m % 2 == 0:
                        nc.scalar.activation(out=dst, in_=src, func=AF.Identity,
                                             bias=bias_sb[p][:, m:m + 1],
                                             scale=1.0)
                    else:
                        nc.vector.tensor_scalar_add(dst, src,
                                                    bias_sb[p][:, m:m + 1])
                if pending:
                    pending.pop(0)()
                    if len(pending) > 6:
                        pending.pop(0)()
                    if len(pending) > 8:
                        pending.pop(0)()
                    if len(pending) > 12:
                        pending.pop(0)()
        if c < NCHUNK - 1:
            pending.extend(chunk_pieces(c))

    # drain the last chunk's attention (B pieces)
    for piece in pending + post_pieces:
        piece()


_PROGRAM = None


def _build():
    global _PROGRAM
    if _PROGRAM is not None:
        return _PROGRAM
    from contextlib import ExitStack

    nc = bacc.Bacc("TRN2", target_bir_lowering=False, debug=False,
                   num_devices=N_CORES)
    with tile.TileContext(nc) as tc:
        with ExitStack() as ctx:
            _emit(nc, tc, ctx)
    nc.compile()
    _PROGRAM = nc
    return nc


def _host_inputs(x, Wq, bq, Wk, bk, Wv, bv):
    """Build the per-core input maps (host-side shard + transpose + cast)."""
    scale = 1.0 / np.sqrt(DG)
    xf = np.ascontiguousarray(x.reshape(-1, D))           # [16384, D]
    assert xf.shape[0] == N_CORES * TC

    bf = ml_dtypes.bfloat16
    shared = {
        "wqT": np.ascontiguousarray((Wq * scale).T).astype(bf),
        "wkT": np.ascontiguousarray(Wk.T).astype(bf),
        "wvT": np.ascontiguousarray(Wv.T).astype(bf),
        "bq": np.ascontiguousarray((bq * scale).reshape(G, DG).T).astype(np.float32),
        "bk": np.ascontiguousarray(bk.reshape(G, DG).T).astype(np.float32),
        "bv": np.ascontiguousarray(bv.reshape(G, DG).T).astype(np.float32),
        "m01": np.ascontiguousarray(np.broadcast_to(
            np.kron(np.ones((G, G), dtype=np.float32),
                    np.eye(8, dtype=np.float32))[:, None, :],
            (P, 4, P))).astype(bf),
        "ident": np.eye(P, dtype=np.float32).astype(bf),
    }
    in_maps = []
    for i in range(N_CORES):
        xi = xf[i * TC:(i + 1) * TC]
        m = dict(shared)
        m["xT"] = np.ascontiguousarray(xi.T).astype(bf)
        in_maps.append(m)
    return in_maps


last_results = None


def _install_ntff_shim():
    """Provide antenv.axon_hooks if the image lacks it (profiling only)."""
    import sys
    try:
        from antenv.axon_hooks import get_axon_ntff_profile_hook  # noqa: F401
        return
    except ImportError:
        pass
    import contextlib
    import ctypes
    import types

    so_path = "/opt/axon/libaxon_pjrt.so"
    hook = None
    if os.path.exists(so_path):
        lib = ctypes.CDLL(so_path)
        if hasattr(lib, "axon_start_nrt_profile"):
            lib.axon_start_nrt_profile.argtypes = [
                ctypes.POINTER(ctypes.c_int64), ctypes.c_size_t]
            lib.axon_start_nrt_profile.restype = ctypes.c_int64
            lib.axon_stop_nrt_profile.argtypes = [ctypes.c_char_p]
            lib.axon_stop_nrt_profile.restype = ctypes.c_int64

            @contextlib.contextmanager
            def _hook(output_dir, device_ids):
                import jax
                jax.devices()
                if device_ids:
                    ids = (ctypes.c_int64 * len(device_ids))(*device_ids)
                    rc = lib.axon_start_nrt_profile(ids, len(device_ids))
                else:
                    rc = lib.axon_start_nrt_profile(None, 0)
                if rc != 0:
                    raise RuntimeError(f"axon_start_nrt_profile rc={rc}")
                try:
                    yield
                finally:
                    n = lib.axon_stop_nrt_profile(str(output_dir).encode())
                    print(f"profile: {n} file(s) written to {output_dir}")

            hook = _hook

    mod = types.ModuleType("antenv.axon_hooks")
    mod.get_axon_ntff_profile_hook = lambda: hook
    mod.set_axon_ntff_profile_hook = lambda h: None
    import antenv
    antenv.axon_hooks = mod
    sys.modules["antenv.axon_hooks"] = mod


def kernel(**inputs):
    global last_results
    nc = _build()
    in_maps = _host_inputs(**inputs)
    trace = bool(os.environ.get("BASS_TRACE"))
    if trace:
        _install_ntff_shim()
    res = run_bass_kernel_spmd(nc, in_maps, list(range(N_CORES)), trace=trace)
    last_results = res
    x = inputs["x"]
    out = np.empty((N_CORES * TC, D), dtype=np.float32)
    for i in range(N_CORES):
        # [dd, tile, sb, g, 32] -> [tile, sb, 32, g, dd] -> [TC, D]
        ot = res.results[i]["outT"].astype(np.float32)
        out[i * TC:(i + 1) * TC] = (
            ot.transpose(1, 2, 4, 3, 0).reshape(TC, D))
    return out.reshape(x.shape)
